# revision 17
# baseline (speedup 1.0000x reference)
"""Cross-attention kernel for Trainium2 (8 NeuronCores, SPMD).

Problem: B=4, LQ=LK=4096, H=256
  query = q @ Wq.T + bq ; keys = k @ Wk.T + bk ; values = v @ Wv.T + bv
  out = softmax(query @ keys.T / sqrt(H)) @ values

Sharding: core i -> batch i//2, query rows (i%2)*2048 .. +2048.
K/V for the batch are replicated across the 2 cores sharing it.

Device algorithm (PE contracts over the partition dim):
  - scores are algebraically refactored:
      s[q,k] = (q M)_q k_k^T + t_q + u_k,  M = Wq.T @ Wk
      t_q cancels in softmax (row-constant), dropped; e^{u_k} is folded
      into the V rows and denominator column on the host, so the device
      exp is bias-free.  qM is host-prepped: NO device projections.
  - scores contract (qM)^T against k^T in fp8 e4m3 DoubleRow: one
    matmul per k-tile contracts all 256 h (2 fp8/cell, 2x ALU rate).
  - scores are computed transposed ([k, q]) so P^T is born k-major.
  - exp on adjacent k-tile PAIRS ([128, 2, 512] PSUM -> bf16 SBUF).
  - P@V is a PRECISION HYBRID tuned to the 2e-2 error budget: k-pairs
    in DRS (6 of each 16) run fp8 DoubleRow -- DVE computes Pt = P - 1
    (bf16 -> fp8; centering keeps fp8 quantization error ~3x smaller
    than quantizing P directly) against host-quantized fp8 V pairs,
    one DR matmul per PAIR per q-window.  The dropped rank-1 term
    ones_q (x) colsum(fp8 vA half) is restored by a host-prepped
    correction pair (bf16, colsum/256 in every slot) contracted against
    an all-ones fp8 stationary as the start=True init of each ctx
    accumulator.  The other 10/16 pairs stay bf16 (P^T stationary
    against bf16 V) -- full fp8 V fails the error budget (2.9e-2).
  - V is augmented with the e^u column ([*, 257]); output column 256 is
    the softmax denominator; context lands in natural [q, h] layout.
    Normalization = per-partition reciprocal + tensor_scalar multiply.
  - score and P@V matmuls interleave per k-pair (P@V lags PLAG pairs)
    so exp (ScalarE) + subtract (DVE) latency hides behind P@V on PE;
    chunk ctx drains qw-major with the normalize fused per q-window.
"""

import os
import sys

import numpy as np

sys.path.insert(0, "/opt/trn_rl_repo")

import ml_dtypes

B, LQ, LK, H = 4, 4096, 4096, 256
P = 128
HO = H // P            # 2 h-tiles
NCORES = 8
NQ = LQ * B // NCORES  # 2048 q rows per core
QC = 512               # q chunk (scores tile width)
NQC = NQ // QC         # 4
QW = QC // P           # 4 q-windows per chunk
KT = LK // P           # 32 k tiles
KP = KT // 2           # 16 k-tile pairs
HA = H + 1             # V augmented with e^u column
PLAG = 4               # P@V lags scores by this many k-PAIRS
NWU = 30               # PE warm-up matmuls (p-state ramp during DMA wait)
SCALE = 1.0 / np.sqrt(np.float32(H))  # 1/16

DRS = (2, 4, 6, 7, 8, 9, 10, 11, 12, 13, 14, 15)  # fp8-DR k-pairs
BFS = tuple(kp for kp in range(KP) if kp not in DRS)
ND2 = 2 * len(DRS)                # fp8 V k-tiles
NB2 = 2 * len(BFS)                # bf16 V k-tiles (+2 corr hi/lo)
DR_IDX = {kp: i for i, kp in enumerate(DRS)}
BF_IDX = {kp: i for i, kp in enumerate(BFS)}

_BF16 = ml_dtypes.bfloat16
_F8 = ml_dtypes.float8_e4m3

_NC_CACHE = None


def _build_nc():
    """Build the single-core Bass program (same program runs SPMD on 8 cores)."""
    import concourse.bass as bass
    import concourse.mybir as mybir
    import concourse.tile as tile
    from concourse import bacc

    f32 = mybir.dt.float32
    bf16 = mybir.dt.bfloat16
    f8 = mybir.dt.float8e4

    nc = bacc.Bacc("TRN2", target_bir_lowering=False, debug=False)

    # All inputs are pre-arranged partition-major on the host so every DMA
    # lands as a few large contiguous runs per partition (descriptor-light).
    kT = nc.declare_dram_parameter("kT", [P, HO, LK], f8, isOutput=False)
    qT = nc.declare_dram_parameter("qT", [P, HO, NQ], f8, isOutput=False)
    vB = nc.declare_dram_parameter("vB", [P, NB2 + 2, HA], bf16,
                                   isOutput=False)
    v8 = nc.declare_dram_parameter("v8", [P, ND2, HA], f8, isOutput=False)
    # bf16 output halves the writeback traffic; host upcasts to f32.
    out = nc.declare_dram_parameter("out", [NQ, H], bf16, isOutput=True)

    qT_r = qT.ap()
    kT_r = kT.ap()
    vB_r = vB.ap()
    v8_r = v8.ap()

    Exp = mybir.ActivationFunctionType.Exp
    DR = mybir.MatmulPerfMode.DoubleRow

    with tile.TileContext(nc) as tc:
        with (
            tc.tile_pool(name="persist", bufs=1) as persist,
        ):
            kraw = persist.tile([P, HO, LK], f8)
            qraw = persist.tile([P, HO, NQ], f8)
            VB_sb = persist.tile([P, NB2 + 2, HA], bf16)
            V8_sb = persist.tile([P, ND2, HA], f8)
            ones_b = persist.tile([P, P], bf16)      # stationary for corr

            nc.vector.memset(ones_b[:], 1.0)

            # DMA issuance costs ~600-800ns per dma_start on the issuing
            # engine's sequencer; issue serially from gpsimd (plus the sync
            # engine for the k front) ordered by first-use time.
            def dk(eng, lo, hi):
                eng.dma_start(kraw[:, :, lo:hi], kT_r[:, :, lo:hi])
            def dq(eng, lo, hi):
                eng.dma_start(qraw[:, :, lo:hi], qT_r[:, :, lo:hi])
            def dvb(eng, lo, hi):
                eng.dma_start(VB_sb[:, lo:hi, :], vB_r[:, lo:hi, :])
            def dv8(eng, lo, hi):
                eng.dma_start(V8_sb[:, lo:hi, :], v8_r[:, lo:hi, :])
            # critical path on sync (its queue frees ~3us before gpsimd,
            # which is stuck behind framework memsets until ~6us); pieces
            # split small because one dma_start binds ONE ~22.5GB/s DMA
            # engine (128KB = 5.7us serial; 2x64KB in parallel = 2.9us)
            dq(nc.sync, 0, QC // 2)
            dq(nc.sync, QC // 2, QC)
            dk(nc.sync, 0, 256)
            dk(nc.sync, 256, 512)
            dvb(nc.sync, NB2, NB2 + 2)     # corr hi/lo: first ctx matmuls
            dk(nc.sync, 512, 1024)
            dv8(nc.gpsimd, 0, ND2)         # fp8 V half
            dk(nc.gpsimd, 1024, 2048)
            dq(nc.gpsimd, QC, 2 * QC)
            dvb(nc.gpsimd, 0, NB2)
            dk(nc.gpsimd, 2048, 3072)
            dk(nc.gpsimd, 3072, 4096)
            dq(nc.gpsimd, 2 * QC, 3 * QC)
            dq(nc.gpsimd, 3 * QC, 4 * QC)

            with (
                tc.tile_pool(name="pt", bufs=10) as ptp,
                tc.tile_pool(name="pt8", bufs=6) as pt8p,
                tc.tile_pool(name="ps_s", bufs=2, space="PSUM") as pss,
                tc.tile_pool(name="ps_ctx", bufs=4, space="PSUM") as psc,
                tc.tile_pool(name="fin", bufs=8) as fin,
            ):
                # PE warm-up: dummy matmuls on a zeroed tile run while the
                # first input DMAs are in flight, so the tensor engine's
                # p-state is fully ramped (~3us of continuous execution)
                # when the real score matmuls start.
                wu = fin.tile([P, P], f8, tag="wu", bufs=1)
                nc.vector.memset(wu[:], 0)
                wups = pss.tile([P, 2, QC], f32, tag="ps_s", name="wups")
                for _ in range(NWU):
                    nc.tensor.matmul(
                        wups[:, 0, :P], wu[:], wu[:], start=True, stop=True,
                    )

                def score_j(qc, kp, j, ps):
                    # one DoubleRow matmul per k-tile: contracts all 256 h
                    kt = 2 * kp + j
                    nc.tensor.matmul(
                        ps[:, j, :],
                        kraw[:, :, kt * P:(kt + 1) * P],
                        qraw[:, :, qc * QC:(qc + 1) * QC],
                        start=True,
                        stop=True,
                        perf_mode=DR,
                    )

                def exp_pair(kp, ps, pts, p8s):
                    pt = ptp.tile([P, 2, QC], bf16, tag="pt")
                    nc.scalar.activation(pt[:], ps[:], Exp, scale=float(SCALE))
                    pts[kp] = pt
                    if kp in DR_IDX:
                        p8 = pt8p.tile([P, 2, QC], f8, tag="pt8")
                        nc.vector.tensor_scalar_add(p8[:], pt[:], -1.0)
                        p8s[kp] = p8

                def corr(ctx, qw):
                    # start=True init: ctx[qw] = ones @ (corr_hi + corr_lo)
                    #                          = colsum of the fp8 V half.
                    # hi/lo bf16 split keeps the restore exact to ~2^-17
                    # (single bf16 rounding of colsum/128 costs ~1e-3 rel).
                    nc.tensor.matmul(
                        ctx[qw][:],
                        ones_b[:],
                        VB_sb[:, NB2, :],
                        start=True,
                        stop=False,
                    )
                    nc.tensor.matmul(
                        ctx[qw][:],
                        ones_b[:],
                        VB_sb[:, NB2 + 1, :],
                        start=False,
                        stop=False,
                    )

                def pv_one(ctx, kp, qw, pts, p8s):
                    last = kp == KP - 1
                    if kp in DR_IDX:
                        i = DR_IDX[kp]
                        nc.tensor.matmul(
                            ctx[qw][:],
                            p8s[kp][:, :, qw * P:(qw + 1) * P],
                            V8_sb[:, 2 * i:2 * i + 2, :],
                            start=False,
                            stop=last,
                            perf_mode=DR,
                        )
                    else:
                        b = BF_IDX[kp]
                        for j in range(2):
                            nc.tensor.matmul(
                                ctx[qw][:],
                                pts[kp][:, j, qw * P:(qw + 1) * P],
                                VB_sb[:, 2 * b + j, :],
                                start=False,
                                stop=last and j == 1,
                            )

                def drain_qw(ctx, pts, p8s, qc, qw):
                    # finish one ctx bank's tail matmuls, then normalize and
                    # write it out, freeing the bank for the next chunk.
                    for kp in range(KP - PLAG, KP):
                        pv_one(ctx, kp, qw, pts, p8s)
                    rec = fin.tile([P, 1], f32, tag="rec")
                    nc.vector.reciprocal(rec[:], ctx[qw][:, H:HA])
                    osb = fin.tile([P, H], bf16, tag="osb")
                    nc.vector.tensor_scalar_mul(osb[:], ctx[qw][:, :H], rec[:])
                    # split the 64KB writeback across two DMA engines so the
                    # last chunk's four drains don't serialize on one queue
                    ro = out.ap()[qc * QC + qw * P:qc * QC + (qw + 1) * P, :]
                    nc.sync.dma_start(ro[:, :H // 2], osb[:, :H // 2])
                    nc.gpsimd.dma_start(ro[:, H // 2:], osb[:, H // 2:])

                # Cross-chunk software pipeline: the previous chunk's tail
                # P@V + normalize is interleaved into the next chunk's first
                # QW score pairs (which have no P@V of their own yet due to
                # PLAG), so PE work per pair is uniform across chunk
                # boundaries.
                prev = None
                for qc in range(NQC):
                    ctx = [psc.tile([P, HA], f32, tag="ps_ctx",
                                    name=f"ctx_{qc}_{qw}")
                           for qw in range(QW)]
                    pts = {}
                    p8s = {}
                    for kp in range(KP):
                        # interleave the pair's two 213ns score matmuls with
                        # the lagged P@V so weight loads hide behind them
                        ps = pss.tile([P, 2, QC], f32, tag="ps_s")
                        pvk = kp - PLAG
                        score_j(qc, kp, 0, ps)
                        if pvk >= 0:
                            pv_one(ctx, pvk, 0, pts, p8s)
                            pv_one(ctx, pvk, 1, pts, p8s)
                        score_j(qc, kp, 1, ps)
                        if pvk >= 0:
                            pv_one(ctx, pvk, 2, pts, p8s)
                            pv_one(ctx, pvk, 3, pts, p8s)
                        exp_pair(kp, ps, pts, p8s)
                        if kp < QW:
                            if prev is not None:
                                drain_qw(prev[0], prev[1], prev[2],
                                         qc - 1, kp)
                            corr(ctx, kp)
                    prev = (ctx, pts, p8s)
                for qw in range(QW):
                    drain_qw(prev[0], prev[1], prev[2], NQC - 1, qw)
    nc.compile()
    return nc


def _get_nc():
    global _NC_CACHE
    if _NC_CACHE is None:
        _NC_CACHE = _build_nc()
    return _NC_CACHE


def _prep_in_maps(q, k, v, Wq, bq, Wk, bk, Wv, bv):
    q = np.asarray(q, np.float32)
    k = np.asarray(k, np.float32)
    v = np.asarray(v, np.float32)
    Wq = np.asarray(Wq, np.float64)
    Wk = np.asarray(Wk, np.float64)
    bq_ = np.asarray(bq, np.float64)
    bk_ = np.asarray(bk, np.float64)
    M = Wq.T @ Wk                       # [h, h~]
    w2v = Wk.T @ bq_                    # [h]
    ccv = float(bq_ @ bk_)
    M32 = M.astype(np.float32)
    Wv32 = np.asarray(Wv, np.float32)
    bv32 = np.asarray(bv, np.float32)
    # k-tile index lists for the two precision halves
    dr_tiles = [2 * kp + j for kp in DRS for j in range(2)]
    bf_tiles = [2 * kp + j for kp in BFS for j in range(2)]
    in_maps = []
    for i in range(NCORES):
        b, half = divmod(i, NCORES // B)
        qm = q[b, half * NQ:(half + 1) * NQ, :] @ M32   # fold M: scores = (qM) k^T
        # partition-major [p, ho, n] with h = ho*128 + p: per-partition data
        # is one contiguous run per ho slice (descriptor-light DMAs).
        qT_i = np.ascontiguousarray(
            qm.T.reshape(HO, P, NQ).transpose(1, 0, 2)).astype(_F8)
        kT_i = np.ascontiguousarray(
            k[b].T.reshape(HO, P, LK).transpose(1, 0, 2)).astype(_F8)
        # e^{u_k}, u_k = (k.(Wk.T bq) + bq.bk)/sqrt(H): folded into V rows
        # and the denominator column so the device exp is bias-free.
        u = (k[b].astype(np.float64) @ w2v + ccv) * float(SCALE)
        eu = np.exp(u).astype(np.float32)
        vA_i = np.empty((LK, HA), np.float32)
        vA_i[:, :H] = (v[b] @ Wv32.T + bv32) * eu[:, None]
        vA_i[:, H] = eu
        # [k, c] -> [p, t, c] with k = t*128 + p
        vA_t = np.ascontiguousarray(
            vA_i.reshape(KT, P, HA).transpose(1, 0, 2)).astype(_BF16)
        # fp8 half: quantize the bf16 values (matches device numerics)
        v8_i = vA_t[:, dr_tiles, :].astype(_F8)
        # correction tile: every slot holds colsum/128 of the UNQUANTIZED
        # vA over the fp8 half, so the rank-1 restore also cancels the
        # P~=1 component of the fp8-V quantization error (the error then
        # enters only through Pt, rms ~0.37, instead of P, rms ~1.18).
        S8 = vA_t[:, dr_tiles, :].astype(np.float32).sum(axis=(0, 1))
        vB_i = np.empty((P, NB2 + 2, HA), _BF16)
        vB_i[:, :NB2, :] = vA_t[:, bf_tiles, :]
        hi = (S8 / 128.0).astype(_BF16)
        lo = ((S8 - hi.astype(np.float32) * 128.0) / 128.0).astype(_BF16)
        vB_i[:, NB2, :] = hi[None, :]
        vB_i[:, NB2 + 1, :] = lo[None, :]
        in_maps.append({
            "qT": qT_i, "kT": kT_i, "vB": vB_i, "v8": v8_i,
        })
    return in_maps


def _install_ntff_hook_shim():
    """The image's antenv lacks axon_hooks; recreate it from the boot recipe
    (ctypes into libaxon_pjrt.so) so trace=True can capture NTFF profiles."""
    import types
    import contextlib
    import ctypes

    if "antenv.axon_hooks" in sys.modules:
        return
    so_path = "/opt/axon/libaxon_pjrt.so"
    hook = None
    if os.path.exists(so_path):
        lib = ctypes.CDLL(so_path)
        if hasattr(lib, "axon_start_nrt_profile"):
            lib.axon_start_nrt_profile.argtypes = [
                ctypes.POINTER(ctypes.c_int64), ctypes.c_size_t]
            lib.axon_start_nrt_profile.restype = ctypes.c_int64
            lib.axon_stop_nrt_profile.argtypes = [ctypes.c_char_p]
            lib.axon_stop_nrt_profile.restype = ctypes.c_int64

            @contextlib.contextmanager
            def _hook(output_dir, device_ids):
                import jax
                jax.devices()
                if device_ids:
                    ids = (ctypes.c_int64 * len(device_ids))(*device_ids)
                    rc = lib.axon_start_nrt_profile(ids, len(device_ids))
                else:
                    rc = lib.axon_start_nrt_profile(None, 0)
                if rc != 0:
                    raise RuntimeError(f"axon_start_nrt_profile rc={rc}")
                try:
                    yield
                finally:
                    n = lib.axon_stop_nrt_profile(str(output_dir).encode())
                    print(f"profile: {n} file(s) written to {output_dir}")

            hook = _hook
    mod = types.ModuleType("antenv.axon_hooks")
    mod.get_axon_ntff_profile_hook = lambda: hook
    mod.set_axon_ntff_profile_hook = lambda h: None
    sys.modules["antenv.axon_hooks"] = mod


def run(inputs, trace=False, trace_cores=None):
    """Run on 8 NeuronCores. Returns (output, BassKernelResults)."""
    from concourse.bass_utils import run_bass_kernel_spmd

    if trace:
        _install_ntff_hook_shim()
    nc = _get_nc()
    in_maps = _prep_in_maps(**inputs)
    res = run_bass_kernel_spmd(
        nc, in_maps, core_ids=list(range(NCORES)),
        trace=trace, trace_cores=trace_cores,
    )
    full = np.empty((B, LQ, H), np.float32)
    for i in range(NCORES):
        b, half = divmod(i, NCORES // B)
        full[b, half * NQ:(half + 1) * NQ, :] = \
            res.results[i]["out"].astype(np.float32)
    return full, res


def kernel(**inputs):
    return run(inputs, trace=False)[0]


# revision 18
# speedup vs baseline: 1.0249x; 1.0249x over previous
"""Cross-attention kernel for Trainium2 (8 NeuronCores, SPMD).

Problem: B=4, LQ=LK=4096, H=256
  query = q @ Wq.T + bq ; keys = k @ Wk.T + bk ; values = v @ Wv.T + bv
  out = softmax(query @ keys.T / sqrt(H)) @ values

Sharding: core i -> batch i//2, query rows (i%2)*2048 .. +2048.
K/V for the batch are replicated across the 2 cores sharing it.

Device algorithm (PE contracts over the partition dim):
  - scores are algebraically refactored:
      s[q,k] = (q M)_q k_k^T + t_q + u_k,  M = Wq.T @ Wk
      t_q cancels in softmax (row-constant), dropped; e^{u_k} is folded
      into the V rows and denominator column on the host, so the device
      exp is bias-free.  qM is host-prepped: NO device projections.
  - scores contract (qM)^T against k^T in fp8 e4m3 DoubleRow: one
    matmul per k-tile contracts all 256 h (2 fp8/cell, 2x ALU rate).
  - scores are computed transposed ([k, q]) so P^T is born k-major.
  - exp on adjacent k-tile PAIRS ([128, 2, 512] PSUM -> bf16 SBUF).
  - P@V is a PRECISION HYBRID tuned to the 2e-2 error budget: k-pairs
    in DRS (6 of each 16) run fp8 DoubleRow -- DVE computes Pt = P - 1
    (bf16 -> fp8; centering keeps fp8 quantization error ~3x smaller
    than quantizing P directly) against host-quantized fp8 V pairs,
    one DR matmul per PAIR per q-window.  The dropped rank-1 term
    ones_q (x) colsum(fp8 vA half) is restored by a host-prepped
    correction pair (bf16, colsum/256 in every slot) contracted against
    an all-ones fp8 stationary as the start=True init of each ctx
    accumulator.  The other 10/16 pairs stay bf16 (P^T stationary
    against bf16 V) -- full fp8 V fails the error budget (2.9e-2).
  - V is augmented with the e^u column ([*, 257]); output column 256 is
    the softmax denominator; context lands in natural [q, h] layout.
    Normalization = per-partition reciprocal + tensor_scalar multiply.
  - score and P@V matmuls interleave per k-pair (P@V lags PLAG pairs)
    so exp (ScalarE) + subtract (DVE) latency hides behind P@V on PE;
    chunk ctx drains qw-major with the normalize fused per q-window.
"""

import os
import sys

import numpy as np

sys.path.insert(0, "/opt/trn_rl_repo")

import ml_dtypes

B, LQ, LK, H = 4, 4096, 4096, 256
P = 128
HO = H // P            # 2 h-tiles
NCORES = 8
NQ = LQ * B // NCORES  # 2048 q rows per core
QC = 512               # q chunk (scores tile width)
NQC = NQ // QC         # 4
QW = QC // P           # 4 q-windows per chunk
KT = LK // P           # 32 k tiles
KP = KT // 2           # 16 k-tile pairs
HA = H + 1             # V augmented with e^u column
PLAG = 4               # P@V lags scores by this many k-PAIRS
NWU = 36               # PE warm-up matmuls (p-state ramp during DMA wait)
SCALE = 1.0 / np.sqrt(np.float32(H))  # 1/16

DRS = (2, 4, 6, 7, 8, 9, 10, 11, 12, 13, 14, 15)  # fp8-DR k-pairs
BFS = tuple(kp for kp in range(KP) if kp not in DRS)
ND2 = 2 * len(DRS)                # fp8 V k-tiles
NB2 = 2 * len(BFS)                # bf16 V k-tiles (+2 corr hi/lo)
DR_IDX = {kp: i for i, kp in enumerate(DRS)}
BF_IDX = {kp: i for i, kp in enumerate(BFS)}

_BF16 = ml_dtypes.bfloat16
_F8 = ml_dtypes.float8_e4m3

_NC_CACHE = None


def _build_nc():
    """Build the single-core Bass program (same program runs SPMD on 8 cores)."""
    import concourse.bass as bass
    import concourse.mybir as mybir
    import concourse.tile as tile
    from concourse import bacc

    f32 = mybir.dt.float32
    bf16 = mybir.dt.bfloat16
    f8 = mybir.dt.float8e4

    nc = bacc.Bacc("TRN2", target_bir_lowering=False, debug=False)

    # All inputs are pre-arranged partition-major on the host so every DMA
    # lands as a few large contiguous runs per partition (descriptor-light).
    kT = nc.declare_dram_parameter("kT", [P, HO, LK], f8, isOutput=False)
    qT = nc.declare_dram_parameter("qT", [P, HO, NQ], f8, isOutput=False)
    vB = nc.declare_dram_parameter("vB", [P, NB2 + 2, HA], bf16,
                                   isOutput=False)
    v8 = nc.declare_dram_parameter("v8", [P, ND2, HA], f8, isOutput=False)
    # bf16 output halves the writeback traffic; host upcasts to f32.
    out = nc.declare_dram_parameter("out", [NQ, H], bf16, isOutput=True)

    qT_r = qT.ap()
    kT_r = kT.ap()
    vB_r = vB.ap()
    v8_r = v8.ap()

    Exp = mybir.ActivationFunctionType.Exp
    DR = mybir.MatmulPerfMode.DoubleRow

    with tile.TileContext(nc) as tc:
        with (
            tc.tile_pool(name="persist", bufs=1) as persist,
        ):
            kraw = persist.tile([P, HO, LK], f8)
            qraw = persist.tile([P, HO, NQ], f8)
            VB_sb = persist.tile([P, NB2 + 2, HA], bf16)
            V8_sb = persist.tile([P, ND2, HA], f8)
            ones_b = persist.tile([P, P], bf16)      # stationary for corr

            nc.vector.memset(ones_b[:], 1.0)

            # DMA issuance costs ~600-800ns per dma_start on the issuing
            # engine's sequencer; issue serially from gpsimd (plus the sync
            # engine for the k front) ordered by first-use time.
            def dk(eng, lo, hi):
                eng.dma_start(kraw[:, :, lo:hi], kT_r[:, :, lo:hi])
            def dq(eng, lo, hi):
                eng.dma_start(qraw[:, :, lo:hi], qT_r[:, :, lo:hi])
            def dvb(eng, lo, hi):
                eng.dma_start(VB_sb[:, lo:hi, :], vB_r[:, lo:hi, :])
            def dv8(eng, lo, hi):
                eng.dma_start(V8_sb[:, lo:hi, :], v8_r[:, lo:hi, :])
            # critical path on sync (its queue frees ~3us before gpsimd,
            # which is stuck behind framework memsets until ~6us); pieces
            # split small because one dma_start binds ONE ~22.5GB/s DMA
            # engine (128KB = 5.7us serial; 2x64KB in parallel = 2.9us)
            dq(nc.sync, 0, QC // 2)
            dk(nc.sync, 0, 256)
            dvb(nc.sync, NB2, NB2 + 2)     # corr hi/lo: first ctx matmuls
            dk(nc.sync, 512, 1024)
            dq(nc.gpsimd, QC // 2, QC)     # q front tail on gpsimd head
            dk(nc.gpsimd, 256, 512)
            dv8(nc.gpsimd, 0, ND2)         # fp8 V half
            dk(nc.gpsimd, 1024, 2048)
            dq(nc.gpsimd, QC, 2 * QC)
            dvb(nc.gpsimd, 0, NB2)
            dk(nc.gpsimd, 2048, 3072)
            dk(nc.gpsimd, 3072, 4096)
            dq(nc.gpsimd, 2 * QC, 3 * QC)
            dq(nc.gpsimd, 3 * QC, 4 * QC)

            with (
                tc.tile_pool(name="pt", bufs=10) as ptp,
                tc.tile_pool(name="pt8", bufs=6) as pt8p,
                tc.tile_pool(name="ps_s", bufs=2, space="PSUM") as pss,
                tc.tile_pool(name="ps_ctx", bufs=4, space="PSUM") as psc,
                tc.tile_pool(name="fin", bufs=8) as fin,
            ):
                # PE warm-up: dummy matmuls on a zeroed tile run while the
                # first input DMAs are in flight, so the tensor engine's
                # p-state is fully ramped (~3us of continuous execution)
                # when the real score matmuls start.
                wu = fin.tile([P, P], f8, tag="wu", bufs=1)
                nc.vector.memset(wu[:], 0)
                wups = pss.tile([P, 2, QC], f32, tag="ps_s", name="wups")
                for _ in range(NWU):
                    nc.tensor.matmul(
                        wups[:, 0, :P], wu[:], wu[:], start=True, stop=True,
                    )

                def score_j(qc, kp, j, ps):
                    # one DoubleRow matmul per k-tile: contracts all 256 h
                    kt = 2 * kp + j
                    nc.tensor.matmul(
                        ps[:, j, :],
                        kraw[:, :, kt * P:(kt + 1) * P],
                        qraw[:, :, qc * QC:(qc + 1) * QC],
                        start=True,
                        stop=True,
                        perf_mode=DR,
                    )

                def exp_pair(kp, ps, pts, p8s):
                    pt = ptp.tile([P, 2, QC], bf16, tag="pt")
                    nc.scalar.activation(pt[:], ps[:], Exp, scale=float(SCALE))
                    pts[kp] = pt
                    if kp in DR_IDX:
                        p8 = pt8p.tile([P, 2, QC], f8, tag="pt8")
                        nc.vector.tensor_scalar_add(p8[:], pt[:], -1.0)
                        p8s[kp] = p8

                def corr(ctx, qw):
                    # start=True init: ctx[qw] = ones @ (corr_hi + corr_lo)
                    #                          = colsum of the fp8 V half.
                    # hi/lo bf16 split keeps the restore exact to ~2^-17
                    # (single bf16 rounding of colsum/128 costs ~1e-3 rel).
                    nc.tensor.matmul(
                        ctx[qw][:],
                        ones_b[:],
                        VB_sb[:, NB2, :],
                        start=True,
                        stop=False,
                    )
                    nc.tensor.matmul(
                        ctx[qw][:],
                        ones_b[:],
                        VB_sb[:, NB2 + 1, :],
                        start=False,
                        stop=False,
                    )

                def pv_one(ctx, kp, qw, pts, p8s):
                    last = kp == KP - 1
                    if kp in DR_IDX:
                        i = DR_IDX[kp]
                        nc.tensor.matmul(
                            ctx[qw][:],
                            p8s[kp][:, :, qw * P:(qw + 1) * P],
                            V8_sb[:, 2 * i:2 * i + 2, :],
                            start=False,
                            stop=last,
                            perf_mode=DR,
                        )
                    else:
                        b = BF_IDX[kp]
                        for j in range(2):
                            nc.tensor.matmul(
                                ctx[qw][:],
                                pts[kp][:, j, qw * P:(qw + 1) * P],
                                VB_sb[:, 2 * b + j, :],
                                start=False,
                                stop=last and j == 1,
                            )

                def drain_qw(ctx, pts, p8s, qc, qw):
                    # finish one ctx bank's tail matmuls, then normalize and
                    # write it out, freeing the bank for the next chunk.
                    for kp in range(KP - PLAG, KP):
                        pv_one(ctx, kp, qw, pts, p8s)
                    rec = fin.tile([P, 1], f32, tag="rec")
                    nc.vector.reciprocal(rec[:], ctx[qw][:, H:HA])
                    osb = fin.tile([P, H], bf16, tag="osb")
                    nc.vector.tensor_scalar_mul(osb[:], ctx[qw][:, :H], rec[:])
                    # split the 64KB writeback across two DMA engines so the
                    # last chunk's four drains don't serialize on one queue
                    ro = out.ap()[qc * QC + qw * P:qc * QC + (qw + 1) * P, :]
                    nc.sync.dma_start(ro[:, :H // 2], osb[:, :H // 2])
                    nc.gpsimd.dma_start(ro[:, H // 2:], osb[:, H // 2:])

                # Cross-chunk software pipeline: the previous chunk's tail
                # P@V + normalize is interleaved into the next chunk's first
                # QW score pairs (which have no P@V of their own yet due to
                # PLAG), so PE work per pair is uniform across chunk
                # boundaries.
                prev = None
                for qc in range(NQC):
                    ctx = [psc.tile([P, HA], f32, tag="ps_ctx",
                                    name=f"ctx_{qc}_{qw}")
                           for qw in range(QW)]
                    pts = {}
                    p8s = {}
                    for kp in range(KP):
                        # interleave the pair's two 213ns score matmuls with
                        # the lagged P@V so weight loads hide behind them
                        ps = pss.tile([P, 2, QC], f32, tag="ps_s")
                        pvk = kp - PLAG
                        score_j(qc, kp, 0, ps)
                        if pvk >= 0:
                            pv_one(ctx, pvk, 0, pts, p8s)
                            pv_one(ctx, pvk, 1, pts, p8s)
                        score_j(qc, kp, 1, ps)
                        if pvk >= 0:
                            pv_one(ctx, pvk, 2, pts, p8s)
                            pv_one(ctx, pvk, 3, pts, p8s)
                        exp_pair(kp, ps, pts, p8s)
                        if kp < QW:
                            if prev is not None:
                                drain_qw(prev[0], prev[1], prev[2],
                                         qc - 1, kp)
                            corr(ctx, kp)
                    prev = (ctx, pts, p8s)
                for qw in range(QW):
                    drain_qw(prev[0], prev[1], prev[2], NQC - 1, qw)
    nc.compile()
    return nc


def _get_nc():
    global _NC_CACHE
    if _NC_CACHE is None:
        _NC_CACHE = _build_nc()
    return _NC_CACHE


def _prep_in_maps(q, k, v, Wq, bq, Wk, bk, Wv, bv):
    q = np.asarray(q, np.float32)
    k = np.asarray(k, np.float32)
    v = np.asarray(v, np.float32)
    Wq = np.asarray(Wq, np.float64)
    Wk = np.asarray(Wk, np.float64)
    bq_ = np.asarray(bq, np.float64)
    bk_ = np.asarray(bk, np.float64)
    M = Wq.T @ Wk                       # [h, h~]
    w2v = Wk.T @ bq_                    # [h]
    ccv = float(bq_ @ bk_)
    M32 = M.astype(np.float32)
    Wv32 = np.asarray(Wv, np.float32)
    bv32 = np.asarray(bv, np.float32)
    # k-tile index lists for the two precision halves
    dr_tiles = [2 * kp + j for kp in DRS for j in range(2)]
    bf_tiles = [2 * kp + j for kp in BFS for j in range(2)]
    in_maps = []
    for i in range(NCORES):
        b, half = divmod(i, NCORES // B)
        qm = q[b, half * NQ:(half + 1) * NQ, :] @ M32   # fold M: scores = (qM) k^T
        # partition-major [p, ho, n] with h = ho*128 + p: per-partition data
        # is one contiguous run per ho slice (descriptor-light DMAs).
        qT_i = np.ascontiguousarray(
            qm.T.reshape(HO, P, NQ).transpose(1, 0, 2)).astype(_F8)
        kT_i = np.ascontiguousarray(
            k[b].T.reshape(HO, P, LK).transpose(1, 0, 2)).astype(_F8)
        # e^{u_k}, u_k = (k.(Wk.T bq) + bq.bk)/sqrt(H): folded into V rows
        # and the denominator column so the device exp is bias-free.
        u = (k[b].astype(np.float64) @ w2v + ccv) * float(SCALE)
        eu = np.exp(u).astype(np.float32)
        vA_i = np.empty((LK, HA), np.float32)
        vA_i[:, :H] = (v[b] @ Wv32.T + bv32) * eu[:, None]
        vA_i[:, H] = eu
        # [k, c] -> [p, t, c] with k = t*128 + p
        vA_t = np.ascontiguousarray(
            vA_i.reshape(KT, P, HA).transpose(1, 0, 2)).astype(_BF16)
        # fp8 half: quantize the bf16 values (matches device numerics)
        v8_i = vA_t[:, dr_tiles, :].astype(_F8)
        # correction tile: every slot holds colsum/128 of the UNQUANTIZED
        # vA over the fp8 half, so the rank-1 restore also cancels the
        # P~=1 component of the fp8-V quantization error (the error then
        # enters only through Pt, rms ~0.37, instead of P, rms ~1.18).
        S8 = vA_t[:, dr_tiles, :].astype(np.float32).sum(axis=(0, 1))
        vB_i = np.empty((P, NB2 + 2, HA), _BF16)
        vB_i[:, :NB2, :] = vA_t[:, bf_tiles, :]
        hi = (S8 / 128.0).astype(_BF16)
        lo = ((S8 - hi.astype(np.float32) * 128.0) / 128.0).astype(_BF16)
        vB_i[:, NB2, :] = hi[None, :]
        vB_i[:, NB2 + 1, :] = lo[None, :]
        in_maps.append({
            "qT": qT_i, "kT": kT_i, "vB": vB_i, "v8": v8_i,
        })
    return in_maps


def _install_ntff_hook_shim():
    """The image's antenv lacks axon_hooks; recreate it from the boot recipe
    (ctypes into libaxon_pjrt.so) so trace=True can capture NTFF profiles."""
    import types
    import contextlib
    import ctypes

    if "antenv.axon_hooks" in sys.modules:
        return
    so_path = "/opt/axon/libaxon_pjrt.so"
    hook = None
    if os.path.exists(so_path):
        lib = ctypes.CDLL(so_path)
        if hasattr(lib, "axon_start_nrt_profile"):
            lib.axon_start_nrt_profile.argtypes = [
                ctypes.POINTER(ctypes.c_int64), ctypes.c_size_t]
            lib.axon_start_nrt_profile.restype = ctypes.c_int64
            lib.axon_stop_nrt_profile.argtypes = [ctypes.c_char_p]
            lib.axon_stop_nrt_profile.restype = ctypes.c_int64

            @contextlib.contextmanager
            def _hook(output_dir, device_ids):
                import jax
                jax.devices()
                if device_ids:
                    ids = (ctypes.c_int64 * len(device_ids))(*device_ids)
                    rc = lib.axon_start_nrt_profile(ids, len(device_ids))
                else:
                    rc = lib.axon_start_nrt_profile(None, 0)
                if rc != 0:
                    raise RuntimeError(f"axon_start_nrt_profile rc={rc}")
                try:
                    yield
                finally:
                    n = lib.axon_stop_nrt_profile(str(output_dir).encode())
                    print(f"profile: {n} file(s) written to {output_dir}")

            hook = _hook
    mod = types.ModuleType("antenv.axon_hooks")
    mod.get_axon_ntff_profile_hook = lambda: hook
    mod.set_axon_ntff_profile_hook = lambda h: None
    sys.modules["antenv.axon_hooks"] = mod


def run(inputs, trace=False, trace_cores=None):
    """Run on 8 NeuronCores. Returns (output, BassKernelResults)."""
    from concourse.bass_utils import run_bass_kernel_spmd

    if trace:
        _install_ntff_hook_shim()
    nc = _get_nc()
    in_maps = _prep_in_maps(**inputs)
    res = run_bass_kernel_spmd(
        nc, in_maps, core_ids=list(range(NCORES)),
        trace=trace, trace_cores=trace_cores,
    )
    full = np.empty((B, LQ, H), np.float32)
    for i in range(NCORES):
        b, half = divmod(i, NCORES // B)
        full[b, half * NQ:(half + 1) * NQ, :] = \
            res.results[i]["out"].astype(np.float32)
    return full, res


def kernel(**inputs):
    return run(inputs, trace=False)[0]


# revision 20
# speedup vs baseline: 1.1490x; 1.1211x over previous
"""Cross-attention kernel for Trainium2 (8 NeuronCores, SPMD).

Problem: B=4, LQ=LK=4096, H=256
  query = q @ Wq.T + bq ; keys = k @ Wk.T + bk ; values = v @ Wv.T + bv
  out = softmax(query @ keys.T / sqrt(H)) @ values

Sharding: core i -> batch i//2, query rows (i%2)*2048 .. +2048.
K/V for the batch are replicated across the 2 cores sharing it.

Device algorithm (PE contracts over the partition dim):
  - scores are algebraically refactored:
      s[q,k] = (q M)_q k_k^T + t_q + u_k,  M = Wq.T @ Wk
      t_q cancels in softmax (row-constant), dropped; e^{u_k} is folded
      into the V rows and denominator column on the host, so the device
      exp is bias-free.  qM is host-prepped: NO device projections.
  - scores contract (qM)^T against k^T in fp8 e4m3 DoubleRow: one
    matmul per k-tile contracts all 256 h (2 fp8/cell, 2x ALU rate).
  - scores are computed transposed ([k, q]) so P^T is born k-major.
  - exp on adjacent k-tile PAIRS ([128, 2, 512] PSUM -> bf16 SBUF).
  - P@V is a PRECISION HYBRID tuned to the 2e-2 error budget: k-pairs
    in DRS (6 of each 16) run fp8 DoubleRow -- DVE computes Pt = P - 1
    (bf16 -> fp8; centering keeps fp8 quantization error ~3x smaller
    than quantizing P directly) against host-quantized fp8 V pairs,
    one DR matmul per PAIR per q-window.  The dropped rank-1 term
    ones_q (x) colsum(fp8 vA half) is restored by a host-prepped
    correction pair (bf16, colsum/256 in every slot) contracted against
    an all-ones fp8 stationary as the start=True init of each ctx
    accumulator.  The other 10/16 pairs stay bf16 (P^T stationary
    against bf16 V) -- full fp8 V fails the error budget (2.9e-2).
  - V is augmented with the e^u column ([*, 257]); output column 256 is
    the softmax denominator; context lands in natural [q, h] layout.
    Normalization = per-partition reciprocal + tensor_scalar multiply.
  - score and P@V matmuls interleave per k-pair (P@V lags PLAG pairs)
    so exp (ScalarE) + subtract (DVE) latency hides behind P@V on PE;
    chunk ctx drains qw-major with the normalize fused per q-window.
"""

import os
import sys

import numpy as np

sys.path.insert(0, "/opt/trn_rl_repo")

import ml_dtypes

B, LQ, LK, H = 4, 4096, 4096, 256
P = 128
HO = H // P            # 2 h-tiles
NCORES = 8
NQ = LQ * B // NCORES  # 2048 q rows per core
QC = 512               # q chunk (scores tile width)
NQC = NQ // QC         # 4
QW = QC // P           # 4 q-windows per chunk
KT = LK // P           # 32 k tiles
KP = KT // 2           # 16 k-tile pairs
HA = H + 1             # V augmented with e^u column
PLAG = 4               # P@V lags scores by this many k-PAIRS
NWU = 36               # PE warm-up matmuls (p-state ramp during DMA wait)
SCALE = 1.0 / np.sqrt(np.float32(H))  # 1/16

DRS = (2, 4, 6, 7, 8, 9, 10, 11, 12, 13, 14, 15)  # fp8-DR k-pairs
BFS = tuple(kp for kp in range(KP) if kp not in DRS)
ND2 = 2 * len(DRS)                # fp8 V k-tiles
NB2 = 2 * len(BFS)                # bf16 V k-tiles (+2 corr hi/lo)
DR_IDX = {kp: i for i, kp in enumerate(DRS)}
BF_IDX = {kp: i for i, kp in enumerate(BFS)}

_BF16 = ml_dtypes.bfloat16
_F8 = ml_dtypes.float8_e4m3

_NC_CACHE = None


def _build_nc():
    """Build the single-core Bass program (same program runs SPMD on 8 cores)."""
    import concourse.bass as bass
    import concourse.mybir as mybir
    import concourse.tile as tile
    from concourse import bacc

    f32 = mybir.dt.float32
    bf16 = mybir.dt.bfloat16
    f8 = mybir.dt.float8e4

    nc = bacc.Bacc("TRN2", target_bir_lowering=False, debug=False)

    # All inputs are pre-arranged partition-major on the host so every DMA
    # lands as a few large contiguous runs per partition (descriptor-light).
    kT = nc.declare_dram_parameter("kT", [P, HO, LK], f8, isOutput=False)
    qT = nc.declare_dram_parameter("qT", [P, HO, NQ], f8, isOutput=False)
    vB = nc.declare_dram_parameter("vB", [P, NB2 + 2, HA], bf16,
                                   isOutput=False)
    v8 = nc.declare_dram_parameter("v8", [P, ND2, HA], f8, isOutput=False)
    # bf16 output halves the writeback traffic; host upcasts to f32.
    out = nc.declare_dram_parameter("out", [NQ, H], bf16, isOutput=True)

    qT_r = qT.ap()
    kT_r = kT.ap()
    vB_r = vB.ap()
    v8_r = v8.ap()

    Exp = mybir.ActivationFunctionType.Exp
    DR = mybir.MatmulPerfMode.DoubleRow

    with tile.TileContext(nc) as tc:
        with (
            tc.tile_pool(name="persist", bufs=1) as persist,
        ):
            kraw = persist.tile([P, HO, LK], f8)
            qraw = persist.tile([P, HO, NQ], f8)
            VB_sb = persist.tile([P, NB2 + 2, HA], bf16)
            V8_sb = persist.tile([P, ND2, HA], f8)
            ones_b = persist.tile([P, P], bf16)      # stationary for corr

            nc.vector.memset(ones_b[:], 1.0)

            # DMA issuance costs ~600-800ns per dma_start on the issuing
            # engine's sequencer; issue serially from gpsimd (plus the sync
            # engine for the k front) ordered by first-use time.
            def dk(eng, lo, hi):
                eng.dma_start(kraw[:, :, lo:hi], kT_r[:, :, lo:hi])
            def dq(eng, lo, hi):
                eng.dma_start(qraw[:, :, lo:hi], qT_r[:, :, lo:hi])
            def dvb(eng, lo, hi):
                eng.dma_start(VB_sb[:, lo:hi, :], vB_r[:, lo:hi, :])
            def dv8(eng, lo, hi):
                eng.dma_start(V8_sb[:, lo:hi, :], v8_r[:, lo:hi, :])
            # critical path on sync (its queue frees ~3us before gpsimd,
            # which is stuck behind framework memsets until ~6us); pieces
            # split small because one dma_start binds ONE ~22.5GB/s DMA
            # engine (128KB = 5.7us serial; 2x64KB in parallel = 2.9us)
            # each issuing queue sustains ~2 concurrent transfers; spread
            # the critical k/q front across sync+scalar first slots
            dq(nc.sync, 0, QC // 2)
            dk(nc.sync, 0, 256)
            dk(nc.sync, 512, 768)
            dk(nc.sync, 768, 1024)
            dvb(nc.sync, NB2, NB2 + 2)     # corr hi/lo: first ctx matmuls
            dq(nc.scalar, QC // 2, QC)     # parallel queue for the q front
            dk(nc.scalar, 256, 512)
            dv8(nc.gpsimd, 0, ND2)         # fp8 V half
            dk(nc.gpsimd, 1024, 2048)
            dq(nc.gpsimd, QC, 2 * QC)
            dvb(nc.gpsimd, 0, NB2)
            dk(nc.gpsimd, 2048, 3072)
            dk(nc.gpsimd, 3072, 4096)
            dq(nc.gpsimd, 2 * QC, 3 * QC)
            dq(nc.gpsimd, 3 * QC, 4 * QC)

            with (
                tc.tile_pool(name="pt", bufs=10) as ptp,
                tc.tile_pool(name="pt8", bufs=6) as pt8p,
                tc.tile_pool(name="ps_s", bufs=2, space="PSUM") as pss,
                tc.tile_pool(name="ps_ctx", bufs=4, space="PSUM") as psc,
                tc.tile_pool(name="fin", bufs=8) as fin,
            ):
                # PE warm-up: dummy matmuls on a zeroed tile run while the
                # first input DMAs are in flight, so the tensor engine's
                # p-state is fully ramped (~3us of continuous execution)
                # when the real score matmuls start.
                wu = fin.tile([P, P], f8, tag="wu", bufs=1)
                nc.vector.memset(wu[:], 0)
                wups = pss.tile([P, 2, QC], f32, tag="ps_s", name="wups")
                for _ in range(NWU):
                    nc.tensor.matmul(
                        wups[:, 0, :P], wu[:], wu[:], start=True, stop=True,
                    )

                def score_j(qc, kp, j, ps):
                    # one DoubleRow matmul per k-tile: contracts all 256 h
                    kt = 2 * kp + j
                    nc.tensor.matmul(
                        ps[:, j, :],
                        kraw[:, :, kt * P:(kt + 1) * P],
                        qraw[:, :, qc * QC:(qc + 1) * QC],
                        start=True,
                        stop=True,
                        perf_mode=DR,
                    )

                def exp_pair(kp, ps, pts, p8s):
                    pt = ptp.tile([P, 2, QC], bf16, tag="pt")
                    nc.scalar.activation(pt[:], ps[:], Exp, scale=float(SCALE))
                    pts[kp] = pt
                    if kp in DR_IDX:
                        p8 = pt8p.tile([P, 2, QC], f8, tag="pt8")
                        nc.vector.tensor_scalar_add(p8[:], pt[:], -1.0)
                        p8s[kp] = p8

                def corr(ctx, qw):
                    # start=True init: ctx[qw] = ones @ (corr_hi + corr_lo)
                    #                          = colsum of the fp8 V half.
                    # hi/lo bf16 split keeps the restore exact to ~2^-17
                    # (single bf16 rounding of colsum/128 costs ~1e-3 rel).
                    nc.tensor.matmul(
                        ctx[qw][:],
                        ones_b[:],
                        VB_sb[:, NB2, :],
                        start=True,
                        stop=False,
                    )
                    nc.tensor.matmul(
                        ctx[qw][:],
                        ones_b[:],
                        VB_sb[:, NB2 + 1, :],
                        start=False,
                        stop=False,
                    )

                def pv_one(ctx, kp, qw, pts, p8s):
                    last = kp == KP - 1
                    if kp in DR_IDX:
                        i = DR_IDX[kp]
                        nc.tensor.matmul(
                            ctx[qw][:],
                            p8s[kp][:, :, qw * P:(qw + 1) * P],
                            V8_sb[:, 2 * i:2 * i + 2, :],
                            start=False,
                            stop=last,
                            perf_mode=DR,
                        )
                    else:
                        b = BF_IDX[kp]
                        for j in range(2):
                            nc.tensor.matmul(
                                ctx[qw][:],
                                pts[kp][:, j, qw * P:(qw + 1) * P],
                                VB_sb[:, 2 * b + j, :],
                                start=False,
                                stop=last and j == 1,
                            )

                def drain_qw(ctx, pts, p8s, qc, qw):
                    # finish one ctx bank's tail matmuls, then normalize and
                    # write it out, freeing the bank for the next chunk.
                    for kp in range(KP - PLAG, KP):
                        pv_one(ctx, kp, qw, pts, p8s)
                    rec = fin.tile([P, 1], f32, tag="rec")
                    nc.vector.reciprocal(rec[:], ctx[qw][:, H:HA])
                    osb = fin.tile([P, H], bf16, tag="osb")
                    nc.vector.tensor_scalar_mul(osb[:], ctx[qw][:, :H], rec[:])
                    # split the 64KB writeback across two DMA engines so the
                    # last chunk's four drains don't serialize on one queue
                    ro = out.ap()[qc * QC + qw * P:qc * QC + (qw + 1) * P, :]
                    nc.sync.dma_start(ro[:, :H // 2], osb[:, :H // 2])
                    nc.gpsimd.dma_start(ro[:, H // 2:], osb[:, H // 2:])

                # Cross-chunk software pipeline: the previous chunk's tail
                # P@V + normalize is interleaved into the next chunk's first
                # QW score pairs (which have no P@V of their own yet due to
                # PLAG), so PE work per pair is uniform across chunk
                # boundaries.
                prev = None
                for qc in range(NQC):
                    ctx = [psc.tile([P, HA], f32, tag="ps_ctx",
                                    name=f"ctx_{qc}_{qw}")
                           for qw in range(QW)]
                    pts = {}
                    p8s = {}
                    for kp in range(KP):
                        # interleave the pair's two 213ns score matmuls with
                        # the lagged P@V so weight loads hide behind them
                        ps = pss.tile([P, 2, QC], f32, tag="ps_s")
                        pvk = kp - PLAG
                        score_j(qc, kp, 0, ps)
                        if pvk >= 0:
                            pv_one(ctx, pvk, 0, pts, p8s)
                            pv_one(ctx, pvk, 1, pts, p8s)
                        score_j(qc, kp, 1, ps)
                        if pvk >= 0:
                            pv_one(ctx, pvk, 2, pts, p8s)
                            pv_one(ctx, pvk, 3, pts, p8s)
                        exp_pair(kp, ps, pts, p8s)
                        if kp < QW:
                            if prev is not None:
                                drain_qw(prev[0], prev[1], prev[2],
                                         qc - 1, kp)
                            corr(ctx, kp)
                    prev = (ctx, pts, p8s)
                for qw in range(QW):
                    drain_qw(prev[0], prev[1], prev[2], NQC - 1, qw)
    nc.compile()
    return nc


def _get_nc():
    global _NC_CACHE
    if _NC_CACHE is None:
        _NC_CACHE = _build_nc()
    return _NC_CACHE


def _prep_in_maps(q, k, v, Wq, bq, Wk, bk, Wv, bv):
    q = np.asarray(q, np.float32)
    k = np.asarray(k, np.float32)
    v = np.asarray(v, np.float32)
    Wq = np.asarray(Wq, np.float64)
    Wk = np.asarray(Wk, np.float64)
    bq_ = np.asarray(bq, np.float64)
    bk_ = np.asarray(bk, np.float64)
    M = Wq.T @ Wk                       # [h, h~]
    w2v = Wk.T @ bq_                    # [h]
    ccv = float(bq_ @ bk_)
    M32 = M.astype(np.float32)
    Wv32 = np.asarray(Wv, np.float32)
    bv32 = np.asarray(bv, np.float32)
    # k-tile index lists for the two precision halves
    dr_tiles = [2 * kp + j for kp in DRS for j in range(2)]
    bf_tiles = [2 * kp + j for kp in BFS for j in range(2)]
    in_maps = []
    for i in range(NCORES):
        b, half = divmod(i, NCORES // B)
        qm = q[b, half * NQ:(half + 1) * NQ, :] @ M32   # fold M: scores = (qM) k^T
        # partition-major [p, ho, n] with h = ho*128 + p: per-partition data
        # is one contiguous run per ho slice (descriptor-light DMAs).
        qT_i = np.ascontiguousarray(
            qm.T.reshape(HO, P, NQ).transpose(1, 0, 2)).astype(_F8)
        kT_i = np.ascontiguousarray(
            k[b].T.reshape(HO, P, LK).transpose(1, 0, 2)).astype(_F8)
        # e^{u_k}, u_k = (k.(Wk.T bq) + bq.bk)/sqrt(H): folded into V rows
        # and the denominator column so the device exp is bias-free.
        u = (k[b].astype(np.float64) @ w2v + ccv) * float(SCALE)
        eu = np.exp(u).astype(np.float32)
        vA_i = np.empty((LK, HA), np.float32)
        vA_i[:, :H] = (v[b] @ Wv32.T + bv32) * eu[:, None]
        vA_i[:, H] = eu
        # [k, c] -> [p, t, c] with k = t*128 + p
        vA_t = np.ascontiguousarray(
            vA_i.reshape(KT, P, HA).transpose(1, 0, 2)).astype(_BF16)
        # fp8 half: quantize the bf16 values (matches device numerics)
        v8_i = vA_t[:, dr_tiles, :].astype(_F8)
        # correction tile: every slot holds colsum/128 of the UNQUANTIZED
        # vA over the fp8 half, so the rank-1 restore also cancels the
        # P~=1 component of the fp8-V quantization error (the error then
        # enters only through Pt, rms ~0.37, instead of P, rms ~1.18).
        S8 = vA_t[:, dr_tiles, :].astype(np.float32).sum(axis=(0, 1))
        vB_i = np.empty((P, NB2 + 2, HA), _BF16)
        vB_i[:, :NB2, :] = vA_t[:, bf_tiles, :]
        hi = (S8 / 128.0).astype(_BF16)
        lo = ((S8 - hi.astype(np.float32) * 128.0) / 128.0).astype(_BF16)
        vB_i[:, NB2, :] = hi[None, :]
        vB_i[:, NB2 + 1, :] = lo[None, :]
        in_maps.append({
            "qT": qT_i, "kT": kT_i, "vB": vB_i, "v8": v8_i,
        })
    return in_maps


def _install_ntff_hook_shim():
    """The image's antenv lacks axon_hooks; recreate it from the boot recipe
    (ctypes into libaxon_pjrt.so) so trace=True can capture NTFF profiles."""
    import types
    import contextlib
    import ctypes

    if "antenv.axon_hooks" in sys.modules:
        return
    so_path = "/opt/axon/libaxon_pjrt.so"
    hook = None
    if os.path.exists(so_path):
        lib = ctypes.CDLL(so_path)
        if hasattr(lib, "axon_start_nrt_profile"):
            lib.axon_start_nrt_profile.argtypes = [
                ctypes.POINTER(ctypes.c_int64), ctypes.c_size_t]
            lib.axon_start_nrt_profile.restype = ctypes.c_int64
            lib.axon_stop_nrt_profile.argtypes = [ctypes.c_char_p]
            lib.axon_stop_nrt_profile.restype = ctypes.c_int64

            @contextlib.contextmanager
            def _hook(output_dir, device_ids):
                import jax
                jax.devices()
                if device_ids:
                    ids = (ctypes.c_int64 * len(device_ids))(*device_ids)
                    rc = lib.axon_start_nrt_profile(ids, len(device_ids))
                else:
                    rc = lib.axon_start_nrt_profile(None, 0)
                if rc != 0:
                    raise RuntimeError(f"axon_start_nrt_profile rc={rc}")
                try:
                    yield
                finally:
                    n = lib.axon_stop_nrt_profile(str(output_dir).encode())
                    print(f"profile: {n} file(s) written to {output_dir}")

            hook = _hook
    mod = types.ModuleType("antenv.axon_hooks")
    mod.get_axon_ntff_profile_hook = lambda: hook
    mod.set_axon_ntff_profile_hook = lambda h: None
    sys.modules["antenv.axon_hooks"] = mod


def run(inputs, trace=False, trace_cores=None):
    """Run on 8 NeuronCores. Returns (output, BassKernelResults)."""
    from concourse.bass_utils import run_bass_kernel_spmd

    if trace:
        _install_ntff_hook_shim()
    nc = _get_nc()
    in_maps = _prep_in_maps(**inputs)
    res = run_bass_kernel_spmd(
        nc, in_maps, core_ids=list(range(NCORES)),
        trace=trace, trace_cores=trace_cores,
    )
    full = np.empty((B, LQ, H), np.float32)
    for i in range(NCORES):
        b, half = divmod(i, NCORES // B)
        full[b, half * NQ:(half + 1) * NQ, :] = \
            res.results[i]["out"].astype(np.float32)
    return full, res


def kernel(**inputs):
    return run(inputs, trace=False)[0]


# revision 21
# speedup vs baseline: 1.2015x; 1.0457x over previous
"""Cross-attention kernel for Trainium2 (8 NeuronCores, SPMD).

Problem: B=4, LQ=LK=4096, H=256
  query = q @ Wq.T + bq ; keys = k @ Wk.T + bk ; values = v @ Wv.T + bv
  out = softmax(query @ keys.T / sqrt(H)) @ values

Sharding: core i -> batch i//2, query rows (i%2)*2048 .. +2048.
K/V for the batch are replicated across the 2 cores sharing it.

Device algorithm (PE contracts over the partition dim):
  - scores are algebraically refactored:
      s[q,k] = (q M)_q k_k^T + t_q + u_k,  M = Wq.T @ Wk
      t_q cancels in softmax (row-constant), dropped; e^{u_k} is folded
      into the V rows and denominator column on the host, so the device
      exp is bias-free.  qM is host-prepped: NO device projections.
  - scores contract (qM)^T against k^T in fp8 e4m3 DoubleRow: one
    matmul per k-tile contracts all 256 h (2 fp8/cell, 2x ALU rate).
  - scores are computed transposed ([k, q]) so P^T is born k-major.
  - exp on adjacent k-tile PAIRS ([128, 2, 512] PSUM -> bf16 SBUF).
  - P@V is a PRECISION HYBRID tuned to the 2e-2 error budget: k-pairs
    in DRS (6 of each 16) run fp8 DoubleRow -- DVE computes Pt = P - 1
    (bf16 -> fp8; centering keeps fp8 quantization error ~3x smaller
    than quantizing P directly) against host-quantized fp8 V pairs,
    one DR matmul per PAIR per q-window.  The dropped rank-1 term
    ones_q (x) colsum(fp8 vA half) is restored by a host-prepped
    correction pair (bf16, colsum/256 in every slot) contracted against
    an all-ones fp8 stationary as the start=True init of each ctx
    accumulator.  The other 10/16 pairs stay bf16 (P^T stationary
    against bf16 V) -- full fp8 V fails the error budget (2.9e-2).
  - V is augmented with the e^u column ([*, 257]); output column 256 is
    the softmax denominator; context lands in natural [q, h] layout.
    Normalization = per-partition reciprocal + tensor_scalar multiply.
  - score and P@V matmuls interleave per k-pair (P@V lags PLAG pairs)
    so exp (ScalarE) + subtract (DVE) latency hides behind P@V on PE;
    chunk ctx drains qw-major with the normalize fused per q-window.
"""

import os
import sys

import numpy as np

sys.path.insert(0, "/opt/trn_rl_repo")

import ml_dtypes

B, LQ, LK, H = 4, 4096, 4096, 256
P = 128
HO = H // P            # 2 h-tiles
NCORES = 8
NQ = LQ * B // NCORES  # 2048 q rows per core
QC = 512               # q chunk (scores tile width)
NQC = NQ // QC         # 4
QW = QC // P           # 4 q-windows per chunk
KT = LK // P           # 32 k tiles
KP = KT // 2           # 16 k-tile pairs
HA = H + 1             # V augmented with e^u column
PLAG = 4               # P@V lags scores by this many k-PAIRS
NWU = 36               # PE warm-up matmuls (p-state ramp during DMA wait)
SCALE = 1.0 / np.sqrt(np.float32(H))  # 1/16

DRS = (2, 4, 6, 7, 8, 9, 10, 11, 12, 13, 14, 15)  # fp8-DR k-pairs
BFS = tuple(kp for kp in range(KP) if kp not in DRS)
ND2 = 2 * len(DRS)                # fp8 V k-tiles
NB2 = 2 * len(BFS)                # bf16 V k-tiles (+2 corr hi/lo)
DR_IDX = {kp: i for i, kp in enumerate(DRS)}
BF_IDX = {kp: i for i, kp in enumerate(BFS)}

_BF16 = ml_dtypes.bfloat16
_F8 = ml_dtypes.float8_e4m3

_NC_CACHE = None


def _build_nc():
    """Build the single-core Bass program (same program runs SPMD on 8 cores)."""
    import concourse.bass as bass
    import concourse.mybir as mybir
    import concourse.tile as tile
    from concourse import bacc

    f32 = mybir.dt.float32
    bf16 = mybir.dt.bfloat16
    f8 = mybir.dt.float8e4

    nc = bacc.Bacc("TRN2", target_bir_lowering=False, debug=False)

    # All inputs are pre-arranged partition-major on the host so every DMA
    # lands as a few large contiguous runs per partition (descriptor-light).
    kT = nc.declare_dram_parameter("kT", [P, HO, LK], f8, isOutput=False)
    qT = nc.declare_dram_parameter("qT", [P, HO, NQ], f8, isOutput=False)
    vB = nc.declare_dram_parameter("vB", [P, NB2 + 2, HA], bf16,
                                   isOutput=False)
    v8 = nc.declare_dram_parameter("v8", [P, ND2, HA], f8, isOutput=False)
    # bf16 output halves the writeback traffic; host upcasts to f32.
    out = nc.declare_dram_parameter("out", [NQ, H], bf16, isOutput=True)

    qT_r = qT.ap()
    kT_r = kT.ap()
    vB_r = vB.ap()
    v8_r = v8.ap()

    Exp = mybir.ActivationFunctionType.Exp
    DR = mybir.MatmulPerfMode.DoubleRow

    with tile.TileContext(nc) as tc:
        with (
            tc.tile_pool(name="persist", bufs=1) as persist,
        ):
            kraw = persist.tile([P, HO, LK], f8)
            qraw = persist.tile([P, HO, NQ], f8)
            VB_sb = persist.tile([P, NB2 + 2, HA], bf16)
            V8_sb = persist.tile([P, ND2, HA], f8)
            ones_b = persist.tile([P, P], bf16)      # stationary for corr

            nc.vector.memset(ones_b[:], 1.0)

            # DMA issuance costs ~600-800ns per dma_start on the issuing
            # engine's sequencer; issue serially from gpsimd (plus the sync
            # engine for the k front) ordered by first-use time.
            def dk(eng, lo, hi):
                eng.dma_start(kraw[:, :, lo:hi], kT_r[:, :, lo:hi])
            def dq(eng, lo, hi):
                eng.dma_start(qraw[:, :, lo:hi], qT_r[:, :, lo:hi])
            def dvb(eng, lo, hi):
                eng.dma_start(VB_sb[:, lo:hi, :], vB_r[:, lo:hi, :])
            def dv8(eng, lo, hi):
                eng.dma_start(V8_sb[:, lo:hi, :], v8_r[:, lo:hi, :])
            # critical path on sync (its queue frees ~3us before gpsimd,
            # which is stuck behind framework memsets until ~6us); pieces
            # split small because one dma_start binds ONE ~22.5GB/s DMA
            # engine (128KB = 5.7us serial; 2x64KB in parallel = 2.9us)
            dq(nc.sync, 0, QC // 2)
            dk(nc.sync, 0, 256)
            dvb(nc.sync, NB2, NB2 + 2)     # corr hi/lo: first ctx matmuls
            dk(nc.sync, 512, 1024)
            dq(nc.scalar, QC // 2, QC)     # parallel queue for the q front
            dk(nc.gpsimd, 256, 512)
            dv8(nc.gpsimd, 0, ND2)         # fp8 V half
            dk(nc.gpsimd, 1024, 2048)
            dq(nc.gpsimd, QC, 2 * QC)
            dvb(nc.gpsimd, 0, NB2)
            dk(nc.gpsimd, 2048, 3072)
            dk(nc.gpsimd, 3072, 4096)
            dq(nc.gpsimd, 2 * QC, 3 * QC)
            dq(nc.gpsimd, 3 * QC, 4 * QC)

            with (
                tc.tile_pool(name="pt", bufs=10) as ptp,
                tc.tile_pool(name="pt8", bufs=6) as pt8p,
                tc.tile_pool(name="ps_s", bufs=2, space="PSUM") as pss,
                tc.tile_pool(name="ps_ctx", bufs=4, space="PSUM") as psc,
                tc.tile_pool(name="fin", bufs=8) as fin,
            ):
                # PE warm-up: dummy matmuls on a zeroed tile run while the
                # first input DMAs are in flight, so the tensor engine's
                # p-state is fully ramped (~3us of continuous execution)
                # when the real score matmuls start.
                wu = fin.tile([P, P], f8, tag="wu", bufs=1)
                nc.vector.memset(wu[:], 0)
                wups = pss.tile([P, 2, QC], f32, tag="ps_s", name="wups")
                for _ in range(NWU):
                    nc.tensor.matmul(
                        wups[:, 0, :P], wu[:], wu[:], start=True, stop=True,
                    )

                def score_j(qc, kp, j, ps):
                    # one DoubleRow matmul per k-tile: contracts all 256 h
                    kt = 2 * kp + j
                    nc.tensor.matmul(
                        ps[:, j, :],
                        kraw[:, :, kt * P:(kt + 1) * P],
                        qraw[:, :, qc * QC:(qc + 1) * QC],
                        start=True,
                        stop=True,
                        perf_mode=DR,
                    )

                def exp_pair(kp, ps, pts, p8s):
                    pt = ptp.tile([P, 2, QC], bf16, tag="pt")
                    nc.scalar.activation(pt[:], ps[:], Exp, scale=float(SCALE))
                    pts[kp] = pt
                    if kp in DR_IDX:
                        p8 = pt8p.tile([P, 2, QC], f8, tag="pt8")
                        nc.vector.tensor_scalar_add(p8[:], pt[:], -1.0)
                        p8s[kp] = p8

                def corr(ctx, qw):
                    # start=True init: ctx[qw] = ones @ (corr_hi + corr_lo)
                    #                          = colsum of the fp8 V half.
                    # hi/lo bf16 split keeps the restore exact to ~2^-17
                    # (single bf16 rounding of colsum/128 costs ~1e-3 rel).
                    nc.tensor.matmul(
                        ctx[qw][:],
                        ones_b[:],
                        VB_sb[:, NB2, :],
                        start=True,
                        stop=False,
                    )
                    nc.tensor.matmul(
                        ctx[qw][:],
                        ones_b[:],
                        VB_sb[:, NB2 + 1, :],
                        start=False,
                        stop=False,
                    )

                def pv_one(ctx, kp, qw, pts, p8s):
                    last = kp == KP - 1
                    if kp in DR_IDX:
                        i = DR_IDX[kp]
                        nc.tensor.matmul(
                            ctx[qw][:],
                            p8s[kp][:, :, qw * P:(qw + 1) * P],
                            V8_sb[:, 2 * i:2 * i + 2, :],
                            start=False,
                            stop=last,
                            perf_mode=DR,
                        )
                    else:
                        b = BF_IDX[kp]
                        for j in range(2):
                            nc.tensor.matmul(
                                ctx[qw][:],
                                pts[kp][:, j, qw * P:(qw + 1) * P],
                                VB_sb[:, 2 * b + j, :],
                                start=False,
                                stop=last and j == 1,
                            )

                def drain_qw(ctx, pts, p8s, qc, qw):
                    # finish one ctx bank's tail matmuls, then normalize and
                    # write it out, freeing the bank for the next chunk.
                    for kp in range(KP - PLAG, KP):
                        pv_one(ctx, kp, qw, pts, p8s)
                    rec = fin.tile([P, 1], f32, tag="rec")
                    nc.vector.reciprocal(rec[:], ctx[qw][:, H:HA])
                    osb = fin.tile([P, H], bf16, tag="osb")
                    nc.vector.tensor_scalar_mul(osb[:], ctx[qw][:, :H], rec[:])
                    # split the 64KB writeback across two DMA engines so the
                    # last chunk's four drains don't serialize on one queue
                    ro = out.ap()[qc * QC + qw * P:qc * QC + (qw + 1) * P, :]
                    nc.sync.dma_start(ro[:, :H // 2], osb[:, :H // 2])
                    nc.gpsimd.dma_start(ro[:, H // 2:], osb[:, H // 2:])

                # Cross-chunk software pipeline: the previous chunk's tail
                # P@V + normalize is interleaved into the next chunk's first
                # QW score pairs (which have no P@V of their own yet due to
                # PLAG), so PE work per pair is uniform across chunk
                # boundaries.
                prev = None
                for qc in range(NQC):
                    ctx = [psc.tile([P, HA], f32, tag="ps_ctx",
                                    name=f"ctx_{qc}_{qw}")
                           for qw in range(QW)]
                    pts = {}
                    p8s = {}
                    for kp in range(KP):
                        # interleave the pair's two 213ns score matmuls with
                        # the lagged P@V so weight loads hide behind them
                        ps = pss.tile([P, 2, QC], f32, tag="ps_s")
                        pvk = kp - PLAG
                        score_j(qc, kp, 0, ps)
                        if pvk >= 0:
                            pv_one(ctx, pvk, 0, pts, p8s)
                            pv_one(ctx, pvk, 1, pts, p8s)
                        score_j(qc, kp, 1, ps)
                        if pvk >= 0:
                            pv_one(ctx, pvk, 2, pts, p8s)
                            pv_one(ctx, pvk, 3, pts, p8s)
                        exp_pair(kp, ps, pts, p8s)
                        if kp < QW:
                            if prev is not None:
                                drain_qw(prev[0], prev[1], prev[2],
                                         qc - 1, kp)
                            corr(ctx, kp)
                    prev = (ctx, pts, p8s)
                for qw in range(QW):
                    drain_qw(prev[0], prev[1], prev[2], NQC - 1, qw)
    nc.compile()
    return nc


def _get_nc():
    global _NC_CACHE
    if _NC_CACHE is None:
        _NC_CACHE = _build_nc()
    return _NC_CACHE


def _prep_in_maps(q, k, v, Wq, bq, Wk, bk, Wv, bv):
    q = np.asarray(q, np.float32)
    k = np.asarray(k, np.float32)
    v = np.asarray(v, np.float32)
    Wq = np.asarray(Wq, np.float64)
    Wk = np.asarray(Wk, np.float64)
    bq_ = np.asarray(bq, np.float64)
    bk_ = np.asarray(bk, np.float64)
    M = Wq.T @ Wk                       # [h, h~]
    w2v = Wk.T @ bq_                    # [h]
    ccv = float(bq_ @ bk_)
    M32 = M.astype(np.float32)
    Wv32 = np.asarray(Wv, np.float32)
    bv32 = np.asarray(bv, np.float32)
    # k-tile index lists for the two precision halves
    dr_tiles = [2 * kp + j for kp in DRS for j in range(2)]
    bf_tiles = [2 * kp + j for kp in BFS for j in range(2)]
    in_maps = []
    for i in range(NCORES):
        b, half = divmod(i, NCORES // B)
        qm = q[b, half * NQ:(half + 1) * NQ, :] @ M32   # fold M: scores = (qM) k^T
        # partition-major [p, ho, n] with h = ho*128 + p: per-partition data
        # is one contiguous run per ho slice (descriptor-light DMAs).
        qT_i = np.ascontiguousarray(
            qm.T.reshape(HO, P, NQ).transpose(1, 0, 2)).astype(_F8)
        kT_i = np.ascontiguousarray(
            k[b].T.reshape(HO, P, LK).transpose(1, 0, 2)).astype(_F8)
        # e^{u_k}, u_k = (k.(Wk.T bq) + bq.bk)/sqrt(H): folded into V rows
        # and the denominator column so the device exp is bias-free.
        u = (k[b].astype(np.float64) @ w2v + ccv) * float(SCALE)
        eu = np.exp(u).astype(np.float32)
        vA_i = np.empty((LK, HA), np.float32)
        vA_i[:, :H] = (v[b] @ Wv32.T + bv32) * eu[:, None]
        vA_i[:, H] = eu
        # [k, c] -> [p, t, c] with k = t*128 + p
        vA_t = np.ascontiguousarray(
            vA_i.reshape(KT, P, HA).transpose(1, 0, 2)).astype(_BF16)
        # fp8 half: quantize the bf16 values (matches device numerics)
        v8_i = vA_t[:, dr_tiles, :].astype(_F8)
        # correction tile: every slot holds colsum/128 of the UNQUANTIZED
        # vA over the fp8 half, so the rank-1 restore also cancels the
        # P~=1 component of the fp8-V quantization error (the error then
        # enters only through Pt, rms ~0.37, instead of P, rms ~1.18).
        S8 = vA_t[:, dr_tiles, :].astype(np.float32).sum(axis=(0, 1))
        vB_i = np.empty((P, NB2 + 2, HA), _BF16)
        vB_i[:, :NB2, :] = vA_t[:, bf_tiles, :]
        hi = (S8 / 128.0).astype(_BF16)
        lo = ((S8 - hi.astype(np.float32) * 128.0) / 128.0).astype(_BF16)
        vB_i[:, NB2, :] = hi[None, :]
        vB_i[:, NB2 + 1, :] = lo[None, :]
        in_maps.append({
            "qT": qT_i, "kT": kT_i, "vB": vB_i, "v8": v8_i,
        })
    return in_maps


def _install_ntff_hook_shim():
    """The image's antenv lacks axon_hooks; recreate it from the boot recipe
    (ctypes into libaxon_pjrt.so) so trace=True can capture NTFF profiles."""
    import types
    import contextlib
    import ctypes

    if "antenv.axon_hooks" in sys.modules:
        return
    so_path = "/opt/axon/libaxon_pjrt.so"
    hook = None
    if os.path.exists(so_path):
        lib = ctypes.CDLL(so_path)
        if hasattr(lib, "axon_start_nrt_profile"):
            lib.axon_start_nrt_profile.argtypes = [
                ctypes.POINTER(ctypes.c_int64), ctypes.c_size_t]
            lib.axon_start_nrt_profile.restype = ctypes.c_int64
            lib.axon_stop_nrt_profile.argtypes = [ctypes.c_char_p]
            lib.axon_stop_nrt_profile.restype = ctypes.c_int64

            @contextlib.contextmanager
            def _hook(output_dir, device_ids):
                import jax
                jax.devices()
                if device_ids:
                    ids = (ctypes.c_int64 * len(device_ids))(*device_ids)
                    rc = lib.axon_start_nrt_profile(ids, len(device_ids))
                else:
                    rc = lib.axon_start_nrt_profile(None, 0)
                if rc != 0:
                    raise RuntimeError(f"axon_start_nrt_profile rc={rc}")
                try:
                    yield
                finally:
                    n = lib.axon_stop_nrt_profile(str(output_dir).encode())
                    print(f"profile: {n} file(s) written to {output_dir}")

            hook = _hook
    mod = types.ModuleType("antenv.axon_hooks")
    mod.get_axon_ntff_profile_hook = lambda: hook
    mod.set_axon_ntff_profile_hook = lambda h: None
    sys.modules["antenv.axon_hooks"] = mod


def run(inputs, trace=False, trace_cores=None):
    """Run on 8 NeuronCores. Returns (output, BassKernelResults)."""
    from concourse.bass_utils import run_bass_kernel_spmd

    if trace:
        _install_ntff_hook_shim()
    nc = _get_nc()
    in_maps = _prep_in_maps(**inputs)
    res = run_bass_kernel_spmd(
        nc, in_maps, core_ids=list(range(NCORES)),
        trace=trace, trace_cores=trace_cores,
    )
    full = np.empty((B, LQ, H), np.float32)
    for i in range(NCORES):
        b, half = divmod(i, NCORES // B)
        full[b, half * NQ:(half + 1) * NQ, :] = \
            res.results[i]["out"].astype(np.float32)
    return full, res


def kernel(**inputs):
    return run(inputs, trace=False)[0]


# revision 23
# speedup vs baseline: 1.2099x; 1.0070x over previous
"""Cross-attention kernel for Trainium2 (8 NeuronCores, SPMD).

Problem: B=4, LQ=LK=4096, H=256
  query = q @ Wq.T + bq ; keys = k @ Wk.T + bk ; values = v @ Wv.T + bv
  out = softmax(query @ keys.T / sqrt(H)) @ values

Sharding: core i -> batch i//2, query rows (i%2)*2048 .. +2048.
K/V for the batch are replicated across the 2 cores sharing it.

Device algorithm (PE contracts over the partition dim):
  - scores are algebraically refactored:
      s[q,k] = (q M)_q k_k^T + t_q + u_k,  M = Wq.T @ Wk
      t_q cancels in softmax (row-constant), dropped; e^{u_k} is folded
      into the V rows and denominator column on the host, so the device
      exp is bias-free.  qM is host-prepped: NO device projections.
  - scores contract (qM)^T against k^T in fp8 e4m3 DoubleRow: one
    matmul per k-tile contracts all 256 h (2 fp8/cell, 2x ALU rate).
  - scores are computed transposed ([k, q]) so P^T is born k-major.
  - exp on adjacent k-tile PAIRS ([128, 2, 512] PSUM -> bf16 SBUF).
  - P@V is a PRECISION HYBRID tuned to the 2e-2 error budget: k-pairs
    in DRS (6 of each 16) run fp8 DoubleRow -- DVE computes Pt = P - 1
    (bf16 -> fp8; centering keeps fp8 quantization error ~3x smaller
    than quantizing P directly) against host-quantized fp8 V pairs,
    one DR matmul per PAIR per q-window.  The dropped rank-1 term
    ones_q (x) colsum(fp8 vA half) is restored by a host-prepped
    correction pair (bf16, colsum/256 in every slot) contracted against
    an all-ones fp8 stationary as the start=True init of each ctx
    accumulator.  The other 10/16 pairs stay bf16 (P^T stationary
    against bf16 V) -- full fp8 V fails the error budget (2.9e-2).
  - V is augmented with the e^u column ([*, 257]); output column 256 is
    the softmax denominator; context lands in natural [q, h] layout.
    Normalization = per-partition reciprocal + tensor_scalar multiply.
  - score and P@V matmuls interleave per k-pair (P@V lags PLAG pairs)
    so exp (ScalarE) + subtract (DVE) latency hides behind P@V on PE;
    chunk ctx drains qw-major with the normalize fused per q-window.
"""

import os
import sys

import numpy as np

sys.path.insert(0, "/opt/trn_rl_repo")

import ml_dtypes

B, LQ, LK, H = 4, 4096, 4096, 256
P = 128
HO = H // P            # 2 h-tiles
NCORES = 8
NQ = LQ * B // NCORES  # 2048 q rows per core
QC = 512               # q chunk (scores tile width)
NQC = NQ // QC         # 4
QW = QC // P           # 4 q-windows per chunk
KT = LK // P           # 32 k tiles
KP = KT // 2           # 16 k-tile pairs
HA = H + 1             # V augmented with e^u column
PLAG = 4               # P@V lags scores by this many k-PAIRS (must be >= QW: corr for ctx[qw] is emitted at kp=qw, before the first pv at kp=PLAG)
NWU = 56               # PE warm-up matmuls (p-state ramp during DMA wait)
SCALE = 1.0 / np.sqrt(np.float32(H))  # 1/16

DRS = (2, 4, 6, 7, 8, 9, 10, 11, 12, 13, 14, 15)  # fp8-DR k-pairs
BFS = tuple(kp for kp in range(KP) if kp not in DRS)
ND2 = 2 * len(DRS)                # fp8 V k-tiles
NB2 = 2 * len(BFS)                # bf16 V k-tiles (+2 corr hi/lo)
DR_IDX = {kp: i for i, kp in enumerate(DRS)}
BF_IDX = {kp: i for i, kp in enumerate(BFS)}

_BF16 = ml_dtypes.bfloat16
_F8 = ml_dtypes.float8_e4m3

_NC_CACHE = None


def _build_nc():
    """Build the single-core Bass program (same program runs SPMD on 8 cores)."""
    import concourse.bass as bass
    import concourse.mybir as mybir
    import concourse.tile as tile
    from concourse import bacc

    f32 = mybir.dt.float32
    bf16 = mybir.dt.bfloat16
    f8 = mybir.dt.float8e4

    nc = bacc.Bacc("TRN2", target_bir_lowering=False, debug=False)

    # All inputs are pre-arranged partition-major on the host so every DMA
    # lands as a few large contiguous runs per partition (descriptor-light).
    kT = nc.declare_dram_parameter("kT", [P, HO, LK], f8, isOutput=False)
    qT = nc.declare_dram_parameter("qT", [P, HO, NQ], f8, isOutput=False)
    vB = nc.declare_dram_parameter("vB", [P, NB2 + 2, HA], bf16,
                                   isOutput=False)
    v8 = nc.declare_dram_parameter("v8", [P, ND2, HA], f8, isOutput=False)
    # bf16 output halves the writeback traffic; host upcasts to f32.
    out = nc.declare_dram_parameter("out", [NQ, H], bf16, isOutput=True)

    qT_r = qT.ap()
    kT_r = kT.ap()
    vB_r = vB.ap()
    v8_r = v8.ap()

    Exp = mybir.ActivationFunctionType.Exp
    DR = mybir.MatmulPerfMode.DoubleRow

    with tile.TileContext(nc) as tc:
        with (
            tc.tile_pool(name="persist", bufs=1) as persist,
        ):
            kraw = persist.tile([P, HO, LK], f8)
            qraw = persist.tile([P, HO, NQ], f8)
            VB_sb = persist.tile([P, NB2 + 2, HA], bf16)
            V8_sb = persist.tile([P, ND2, HA], f8)
            ones_b = persist.tile([P, P], bf16)      # stationary for corr

            nc.vector.memset(ones_b[:], 1.0)

            # DMA issuance costs ~600-800ns per dma_start on the issuing
            # engine's sequencer; issue serially from gpsimd (plus the sync
            # engine for the k front) ordered by first-use time.
            def dk(eng, lo, hi):
                eng.dma_start(kraw[:, :, lo:hi], kT_r[:, :, lo:hi])
            def dq(eng, lo, hi):
                eng.dma_start(qraw[:, :, lo:hi], qT_r[:, :, lo:hi])
            def dvb(eng, lo, hi):
                eng.dma_start(VB_sb[:, lo:hi, :], vB_r[:, lo:hi, :])
            def dv8(eng, lo, hi):
                eng.dma_start(V8_sb[:, lo:hi, :], v8_r[:, lo:hi, :])
            # critical path on sync (its queue frees ~3us before gpsimd,
            # which is stuck behind framework memsets until ~6us); pieces
            # split small because one dma_start binds ONE ~22.5GB/s DMA
            # engine (128KB = 5.7us serial; 2x64KB in parallel = 2.9us)
            dq(nc.sync, 0, QC // 2)
            dk(nc.sync, 0, 256)
            dvb(nc.sync, NB2, NB2 + 2)     # corr hi/lo: first ctx matmuls
            dk(nc.sync, 512, 1024)
            dq(nc.scalar, QC // 2, QC)     # parallel queue for the q front
            dk(nc.gpsimd, 256, 512)
            dv8(nc.gpsimd, 0, ND2)         # fp8 V half
            dk(nc.gpsimd, 1024, 2048)
            dq(nc.gpsimd, QC, 2 * QC)
            dvb(nc.gpsimd, 0, NB2)
            dk(nc.gpsimd, 2048, 3072)
            dk(nc.gpsimd, 3072, 4096)
            dq(nc.gpsimd, 2 * QC, 3 * QC)
            dq(nc.gpsimd, 3 * QC, 4 * QC)

            with (
                tc.tile_pool(name="pt", bufs=10) as ptp,
                tc.tile_pool(name="pt8", bufs=6) as pt8p,
                tc.tile_pool(name="ps_s", bufs=2, space="PSUM") as pss,
                tc.tile_pool(name="ps_ctx", bufs=4, space="PSUM") as psc,
                tc.tile_pool(name="fin", bufs=8) as fin,
            ):
                # PE warm-up: dummy matmuls on a zeroed tile run while the
                # first input DMAs are in flight, so the tensor engine's
                # p-state is fully ramped (~3us of continuous execution)
                # when the real score matmuls start.
                wu = fin.tile([P, P], f8, tag="wu", bufs=1)
                nc.vector.memset(wu[:], 0)
                wups = pss.tile([P, 2, QC], f32, tag="ps_s", name="wups")
                for _ in range(NWU):
                    nc.tensor.matmul(
                        wups[:, 0, :P], wu[:], wu[:], start=True, stop=True,
                    )

                def score_j(qc, kp, j, ps):
                    # one DoubleRow matmul per k-tile: contracts all 256 h
                    kt = 2 * kp + j
                    nc.tensor.matmul(
                        ps[:, j, :],
                        kraw[:, :, kt * P:(kt + 1) * P],
                        qraw[:, :, qc * QC:(qc + 1) * QC],
                        start=True,
                        stop=True,
                        perf_mode=DR,
                    )

                def exp_pair(kp, ps, pts, p8s):
                    pt = ptp.tile([P, 2, QC], bf16, tag="pt")
                    nc.scalar.activation(pt[:], ps[:], Exp, scale=float(SCALE))
                    pts[kp] = pt
                    if kp in DR_IDX:
                        p8 = pt8p.tile([P, 2, QC], f8, tag="pt8")
                        nc.vector.tensor_scalar_add(p8[:], pt[:], -1.0)
                        p8s[kp] = p8

                def corr(ctx, qw):
                    # start=True init: ctx[qw] = ones @ (corr_hi + corr_lo)
                    #                          = colsum of the fp8 V half.
                    # hi/lo bf16 split keeps the restore exact to ~2^-17
                    # (single bf16 rounding of colsum/128 costs ~1e-3 rel).
                    nc.tensor.matmul(
                        ctx[qw][:],
                        ones_b[:],
                        VB_sb[:, NB2, :],
                        start=True,
                        stop=False,
                    )
                    nc.tensor.matmul(
                        ctx[qw][:],
                        ones_b[:],
                        VB_sb[:, NB2 + 1, :],
                        start=False,
                        stop=False,
                    )

                def pv_one(ctx, kp, qw, pts, p8s):
                    last = kp == KP - 1
                    if kp in DR_IDX:
                        i = DR_IDX[kp]
                        nc.tensor.matmul(
                            ctx[qw][:],
                            p8s[kp][:, :, qw * P:(qw + 1) * P],
                            V8_sb[:, 2 * i:2 * i + 2, :],
                            start=False,
                            stop=last,
                            perf_mode=DR,
                        )
                    else:
                        b = BF_IDX[kp]
                        for j in range(2):
                            nc.tensor.matmul(
                                ctx[qw][:],
                                pts[kp][:, j, qw * P:(qw + 1) * P],
                                VB_sb[:, 2 * b + j, :],
                                start=False,
                                stop=last and j == 1,
                            )

                def drain_qw(ctx, pts, p8s, qc, qw):
                    # finish one ctx bank's tail matmuls, then normalize and
                    # write it out, freeing the bank for the next chunk.
                    for kp in range(KP - PLAG, KP):
                        pv_one(ctx, kp, qw, pts, p8s)
                    rec = fin.tile([P, 1], f32, tag="rec")
                    nc.vector.reciprocal(rec[:], ctx[qw][:, H:HA])
                    osb = fin.tile([P, H], bf16, tag="osb")
                    nc.vector.tensor_scalar_mul(osb[:], ctx[qw][:, :H], rec[:])
                    # split the 64KB writeback across two DMA engines so the
                    # last chunk's four drains don't serialize on one queue
                    ro = out.ap()[qc * QC + qw * P:qc * QC + (qw + 1) * P, :]
                    nc.sync.dma_start(ro[:, :H // 2], osb[:, :H // 2])
                    nc.gpsimd.dma_start(ro[:, H // 2:], osb[:, H // 2:])

                # Cross-chunk software pipeline: the previous chunk's tail
                # P@V + normalize is interleaved into the next chunk's first
                # QW score pairs (which have no P@V of their own yet due to
                # PLAG), so PE work per pair is uniform across chunk
                # boundaries.
                prev = None
                for qc in range(NQC):
                    ctx = [psc.tile([P, HA], f32, tag="ps_ctx",
                                    name=f"ctx_{qc}_{qw}")
                           for qw in range(QW)]
                    pts = {}
                    p8s = {}
                    for kp in range(KP):
                        # interleave the pair's two 213ns score matmuls with
                        # the lagged P@V so weight loads hide behind them
                        ps = pss.tile([P, 2, QC], f32, tag="ps_s")
                        pvk = kp - PLAG
                        score_j(qc, kp, 0, ps)
                        if pvk >= 0:
                            pv_one(ctx, pvk, 0, pts, p8s)
                            pv_one(ctx, pvk, 1, pts, p8s)
                        score_j(qc, kp, 1, ps)
                        if pvk >= 0:
                            pv_one(ctx, pvk, 2, pts, p8s)
                            pv_one(ctx, pvk, 3, pts, p8s)
                        exp_pair(kp, ps, pts, p8s)
                        if kp < QW:
                            if prev is not None:
                                drain_qw(prev[0], prev[1], prev[2],
                                         qc - 1, kp)
                            corr(ctx, kp)
                    prev = (ctx, pts, p8s)
                for qw in range(QW):
                    drain_qw(prev[0], prev[1], prev[2], NQC - 1, qw)
    nc.compile()
    return nc


def _get_nc():
    global _NC_CACHE
    if _NC_CACHE is None:
        _NC_CACHE = _build_nc()
    return _NC_CACHE


def _prep_in_maps(q, k, v, Wq, bq, Wk, bk, Wv, bv):
    q = np.asarray(q, np.float32)
    k = np.asarray(k, np.float32)
    v = np.asarray(v, np.float32)
    Wq = np.asarray(Wq, np.float64)
    Wk = np.asarray(Wk, np.float64)
    bq_ = np.asarray(bq, np.float64)
    bk_ = np.asarray(bk, np.float64)
    M = Wq.T @ Wk                       # [h, h~]
    w2v = Wk.T @ bq_                    # [h]
    ccv = float(bq_ @ bk_)
    M32 = M.astype(np.float32)
    Wv32 = np.asarray(Wv, np.float32)
    bv32 = np.asarray(bv, np.float32)
    # k-tile index lists for the two precision halves
    dr_tiles = [2 * kp + j for kp in DRS for j in range(2)]
    bf_tiles = [2 * kp + j for kp in BFS for j in range(2)]
    in_maps = []
    for i in range(NCORES):
        b, half = divmod(i, NCORES // B)
        qm = q[b, half * NQ:(half + 1) * NQ, :] @ M32   # fold M: scores = (qM) k^T
        # partition-major [p, ho, n] with h = ho*128 + p: per-partition data
        # is one contiguous run per ho slice (descriptor-light DMAs).
        qT_i = np.ascontiguousarray(
            qm.T.reshape(HO, P, NQ).transpose(1, 0, 2)).astype(_F8)
        kT_i = np.ascontiguousarray(
            k[b].T.reshape(HO, P, LK).transpose(1, 0, 2)).astype(_F8)
        # e^{u_k}, u_k = (k.(Wk.T bq) + bq.bk)/sqrt(H): folded into V rows
        # and the denominator column so the device exp is bias-free.
        u = (k[b].astype(np.float64) @ w2v + ccv) * float(SCALE)
        eu = np.exp(u).astype(np.float32)
        vA_i = np.empty((LK, HA), np.float32)
        vA_i[:, :H] = (v[b] @ Wv32.T + bv32) * eu[:, None]
        vA_i[:, H] = eu
        # [k, c] -> [p, t, c] with k = t*128 + p
        vA_t = np.ascontiguousarray(
            vA_i.reshape(KT, P, HA).transpose(1, 0, 2)).astype(_BF16)
        # fp8 half: quantize the bf16 values (matches device numerics)
        v8_i = vA_t[:, dr_tiles, :].astype(_F8)
        # correction tile: every slot holds colsum/128 of the UNQUANTIZED
        # vA over the fp8 half, so the rank-1 restore also cancels the
        # P~=1 component of the fp8-V quantization error (the error then
        # enters only through Pt, rms ~0.37, instead of P, rms ~1.18).
        S8 = vA_t[:, dr_tiles, :].astype(np.float32).sum(axis=(0, 1))
        vB_i = np.empty((P, NB2 + 2, HA), _BF16)
        vB_i[:, :NB2, :] = vA_t[:, bf_tiles, :]
        hi = (S8 / 128.0).astype(_BF16)
        lo = ((S8 - hi.astype(np.float32) * 128.0) / 128.0).astype(_BF16)
        vB_i[:, NB2, :] = hi[None, :]
        vB_i[:, NB2 + 1, :] = lo[None, :]
        in_maps.append({
            "qT": qT_i, "kT": kT_i, "vB": vB_i, "v8": v8_i,
        })
    return in_maps


def _install_ntff_hook_shim():
    """The image's antenv lacks axon_hooks; recreate it from the boot recipe
    (ctypes into libaxon_pjrt.so) so trace=True can capture NTFF profiles."""
    import types
    import contextlib
    import ctypes

    if "antenv.axon_hooks" in sys.modules:
        return
    so_path = "/opt/axon/libaxon_pjrt.so"
    hook = None
    if os.path.exists(so_path):
        lib = ctypes.CDLL(so_path)
        if hasattr(lib, "axon_start_nrt_profile"):
            lib.axon_start_nrt_profile.argtypes = [
                ctypes.POINTER(ctypes.c_int64), ctypes.c_size_t]
            lib.axon_start_nrt_profile.restype = ctypes.c_int64
            lib.axon_stop_nrt_profile.argtypes = [ctypes.c_char_p]
            lib.axon_stop_nrt_profile.restype = ctypes.c_int64

            @contextlib.contextmanager
            def _hook(output_dir, device_ids):
                import jax
                jax.devices()
                if device_ids:
                    ids = (ctypes.c_int64 * len(device_ids))(*device_ids)
                    rc = lib.axon_start_nrt_profile(ids, len(device_ids))
                else:
                    rc = lib.axon_start_nrt_profile(None, 0)
                if rc != 0:
                    raise RuntimeError(f"axon_start_nrt_profile rc={rc}")
                try:
                    yield
                finally:
                    n = lib.axon_stop_nrt_profile(str(output_dir).encode())
                    print(f"profile: {n} file(s) written to {output_dir}")

            hook = _hook
    mod = types.ModuleType("antenv.axon_hooks")
    mod.get_axon_ntff_profile_hook = lambda: hook
    mod.set_axon_ntff_profile_hook = lambda h: None
    sys.modules["antenv.axon_hooks"] = mod


def run(inputs, trace=False, trace_cores=None):
    """Run on 8 NeuronCores. Returns (output, BassKernelResults)."""
    from concourse.bass_utils import run_bass_kernel_spmd

    if trace:
        _install_ntff_hook_shim()
    nc = _get_nc()
    in_maps = _prep_in_maps(**inputs)
    res = run_bass_kernel_spmd(
        nc, in_maps, core_ids=list(range(NCORES)),
        trace=trace, trace_cores=trace_cores,
    )
    full = np.empty((B, LQ, H), np.float32)
    for i in range(NCORES):
        b, half = divmod(i, NCORES // B)
        full[b, half * NQ:(half + 1) * NQ, :] = \
            res.results[i]["out"].astype(np.float32)
    return full, res


def kernel(**inputs):
    return run(inputs, trace=False)[0]
